# revision 20
# baseline (speedup 1.0000x reference)
"""Trainium2 Bass kernel for cross-attention (b=2, n=m=2048, dim=1024, 16 heads x 64)
with QK-RMSNorm and rotate-half RoPE (float positions), distributed over 8 NeuronCores.

Sharding: core c handles batch b = c//4 and head group hg = c%4 (4 heads each).
Wq/Wkv are column-sharded by head, Wo row-sharded; each core emits a partial
[2048, 1024] output which the host sums over the 4 cores of each batch (the
row-parallel all-reduce done at unshard time).

v2 over the session-1 baseline (249.7us):
  - input DMAs issued in (token-block, kc) chunk order so the KV projection's
    first matmuls start ~2-3us in (was ~26us idle + cold-clock penalty)
  - AV uses column-tiled concurrent M=64 matmul pairs (head pair shares one
    PSUM bank, full PE width) instead of M=65 augmented-V at half width
  - softmax denominator via DVE mc-axis reduce of P + gpsimd
    partition_all_reduce (result broadcast to all partitions) + DVE fast
    reciprocal; no ones-row, no partition_broadcast
  - projection drain copies (K/Q/V psum->sbuf) moved ScalarE -> gpsimd,
    squares moved ScalarE -> DVE: ScalarE keeps only the exp stream + trig
  - norm/rope tails grouped G=8 (halves DVE instruction count)
  - Wo drains via gpsimd, freeing DVE
"""

import math
import os

import numpy as np

B, N, DIM, H, HD = 2, 2048, 1024, 16, 64
NCORES = 8
HPC = 4  # heads per core
QD = HPC * HD  # 256
P = 128
NT = N // P  # 16 token chunks
KC = DIM // P  # 8 contraction chunks
QB = 4  # q blocks of 512
QW = N // QB  # 512
G = 8  # token chunks per norm/rope tail group
ROPE_THETA = 10000.0
EPS = float(np.finfo(np.float32).eps)

_CACHE = {}
LAST_RESULTS = None


def _build_trig(nc, tc, pool, scr, pos_dram, w_sb, invf_sb, consts, tag):
    """sin/cos tables with RMSNorm-weight w folded in. Returns (cw1, cw2, sw1, sw2),
    each [P, NT, 32] bf16: cw1=cos*w[0:32], cw2=cos*w[32:64], sw1=sin*w[0:32],
    sw2=sin*w[32:64]."""
    import concourse.bass as bass
    from concourse import mybir

    f32 = mybir.dt.float32
    bf16 = mybir.dt.bfloat16
    AF = mybir.ActivationFunctionType
    ALU = mybir.AluOpType
    INV2PI, MAGIC, C1, C2, C3 = consts

    pos_sb = scr.tile([P, NT], f32, tag=f"pos{tag}")
    nc.sync.dma_start(out=pos_sb, in_=pos_dram.rearrange("(c p) -> p c", p=P))
    ang = scr.tile([P, NT * 32], f32, tag=f"ang{tag}")
    nc.vector.tensor_tensor(
        ang.rearrange("p (t j) -> p t j", j=32),
        pos_sb[:, :, None].to_broadcast([P, NT, 32]),
        invf_sb[:, None, :].to_broadcast([P, NT, 32]),
        ALU.mult,
    )
    # round(ang / 2pi) via magic-number rounding
    kf = scr.tile([P, NT * 32], f32, tag=f"kf{tag}")
    nc.vector.tensor_scalar(kf, ang, float(INV2PI), float(MAGIC), ALU.mult, ALU.add)
    nc.vector.tensor_scalar(kf, kf, float(MAGIC), None, ALU.subtract)
    angr = scr.tile([P, NT * 32], f32, tag=f"angr{tag}")
    nc.vector.cody_waite_cascade(
        out=angr, x=ang, k=kf, c1=float(C1), c2=float(C2), c3=float(C3)
    )
    # cos argument: wrap(angr + pi/2) into [-pi, pi]
    nc.vector.add_range_wrap(
        out=kf, in_=angr, shift=math.pi / 2, bound=math.pi, period=2 * math.pi
    )
    sint = scr.tile([P, NT, 32], f32, tag=f"sin{tag}")
    cost = scr.tile([P, NT, 32], f32, tag=f"cos{tag}")
    nc.scalar.activation(sint.rearrange("p t j -> p (t j)"), angr, AF.Sin)
    nc.scalar.activation(cost.rearrange("p t j -> p (t j)"), kf, AF.Sin)

    tabs = []
    for name, trig, wlo in (
        ("cw1", cost, True),
        ("cw2", cost, False),
        ("sw1", sint, True),
        ("sw2", sint, False),
    ):
        t = pool.tile([P, NT, 32], bf16, tag=f"{name}{tag}")
        wsl = w_sb[:, 0:32] if wlo else w_sb[:, 32:64]
        nc.vector.tensor_tensor(
            t, trig, wsl[:, None, :].to_broadcast([P, NT, 32]), ALU.mult
        )
        tabs.append(t)
    return tabs


def _tail_group(nc, g, ss, rsq, c, xnat, tabs, xTg, nm, pools, dramp, eps64, gg):
    """After projection chunks gg*g..gg*g+gg-1: rsqrt (DVE fast-inverse-sqrt, 2 NR
    steps), normalize+rope the group, bounce to DRAM, transposed-load into
    xTg[half][(gg//4)*g + sub]."""
    from concourse import mybir

    f32 = mybir.dt.float32
    i32 = mybir.dt.int32
    bf16 = mybir.dt.bfloat16
    ALU = mybir.AluOpType
    acts, ascr = pools
    cw1, cw2, sw1, sw2 = tabs
    MAGIC = 0x5F3759DF
    gs = slice(gg * g, gg * g + gg)

    xg = ascr.tile([P, gg, HPC], f32, tag=f"rsx{gg}", name="rsx")
    yg = ascr.tile([P, gg, HPC], f32, tag=f"rsy{gg}", name="rsy")
    tg = ascr.tile([P, gg, HPC], f32, tag=f"rst{gg}", name="rst")
    nc.vector.tensor_scalar_add(xg, ss[:, gs], eps64)
    nc.vector.tensor_scalar(
        tg.bitcast(i32), xg.bitcast(i32), 1, None, ALU.arith_shift_right
    )
    nc.vector.tensor_scalar(
        yg.bitcast(i32), tg.bitcast(i32), -1, MAGIC, ALU.mult, ALU.add
    )
    nc.vector.tensor_tensor(tg, yg, yg, ALU.mult)
    nc.vector.tensor_tensor(tg, tg, xg, ALU.mult)
    nc.vector.tensor_scalar(tg, tg, -0.5, 1.5, ALU.mult, ALU.add)
    nc.vector.tensor_tensor(yg, yg, tg, ALU.mult)
    nc.vector.tensor_tensor(tg, yg, yg, ALU.mult)
    nc.vector.tensor_tensor(tg, tg, xg, ALU.mult)
    nc.vector.tensor_scalar(tg, tg, -0.5 * c, 1.5 * c, ALU.mult, ALU.add)
    nc.vector.tensor_tensor(rsq[:, gs], yg, tg, ALU.mult)

    xb = ascr.tile([P, gg, HPC, HD], bf16, tag=f"xb{nm}", name=f"xb{nm}")
    xhat = ascr.tile([P, gg, HPC, 2, 32], bf16, tag=f"xh{nm}", name=f"xh{nm}")
    x4 = xnat.rearrange("p t (h d) -> p t h d", h=HPC)
    nc.vector.tensor_tensor(
        xb,
        x4[:, gs],
        rsq[:, gs, :, None].to_broadcast([P, gg, HPC, HD]),
        ALU.mult,
    )
    x1 = xb[:, :, :, 0:32]
    x2 = xb[:, :, :, 32:64]
    sh4 = [P, gg, HPC, 32]

    def bc(t):
        return t[:, gs, None, :].to_broadcast(sh4)

    a = ascr.tile(sh4, bf16, tag=f"ra{gg}", name="ra")
    b = ascr.tile(sh4, bf16, tag=f"rb{gg}", name="rb")
    nc.vector.tensor_tensor(a, x1, bc(cw1), ALU.mult)
    nc.vector.tensor_tensor(b, x2, bc(sw2), ALU.mult)
    nc.vector.tensor_sub(xhat[:, :, :, 0, :], a, b)
    a2 = ascr.tile(sh4, bf16, tag=f"ra{gg}", name="ra2")
    b2 = ascr.tile(sh4, bf16, tag=f"rb{gg}", name="rb2")
    nc.vector.tensor_tensor(a2, x1, bc(sw1), ALU.mult)
    nc.vector.tensor_tensor(b2, x2, bc(cw2), ALU.mult)
    nc.vector.tensor_add(xhat[:, :, :, 1, :], a2, b2)

    scr = dramp.tile([gg * P, QD], bf16, tag=f"scr{nm}{g}", name=f"scr{nm}{g}")
    nc.sync.dma_start(
        out=scr.rearrange("(c p) d -> p c d", p=P),
        in_=xhat.rearrange("p t h two j -> p t (h two j)"),
    )
    nsub = (gg * P) // QW
    for half in range(2):
        for sub in range(nsub):
            nc.sync.dma_start(
                out=xTg[half][(gg // 4) * g + sub],
                in_=scr[sub * QW : (sub + 1) * QW, half * P : (half + 1) * P],
                transpose=True,
            )


def _build():
    import concourse.bass as bass
    import concourse.tile as tile
    from concourse import bacc, bass_isa, mybir

    f32 = mybir.dt.float32
    bf16 = mybir.dt.bfloat16
    AF = mybir.ActivationFunctionType
    ALU = mybir.AluOpType

    nc = bacc.Bacc(
        "TRN2", target_bir_lowering=False, debug=False, num_devices=NCORES
    )

    tgt_t = nc.dram_tensor("tgt_t", [DIM, N], bf16, kind="ExternalInput").ap()
    src_t = nc.dram_tensor("src_t", [DIM, N], bf16, kind="ExternalInput").ap()
    wq_d = nc.dram_tensor("wq", [DIM, QD], bf16, kind="ExternalInput").ap()
    wkv_d = nc.dram_tensor("wkv", [DIM, 2 * QD], bf16, kind="ExternalInput").ap()
    wo_d = nc.dram_tensor("wo", [QD, DIM], bf16, kind="ExternalInput").ap()
    tpos = nc.dram_tensor("tpos", [N], f32, kind="ExternalInput").ap()
    spos = nc.dram_tensor("spos", [N], f32, kind="ExternalInput").ap()
    qw_d = nc.dram_tensor("qw", [HD], f32, kind="ExternalInput").ap()
    kw_d = nc.dram_tensor("kw", [HD], f32, kind="ExternalInput").ap()
    out_d = nc.dram_tensor("out", [N, DIM], f32, kind="ExternalOutput").ap()

    invf_np = np.float32(ROPE_THETA) ** (
        -np.arange(0, HD, 2, dtype=np.float32) / np.float32(HD)
    )
    invf_dram = nc.inline_tensor(invf_np.astype(np.float32), "invf").ap()

    TWO_PI = 2 * math.pi
    C1 = np.float32(6.28125)
    C2 = np.float32(TWO_PI - float(C1))
    C3 = np.float32(TWO_PI - float(C1) - float(C2))
    MAGIC = np.float32(1.5 * 2**23)
    INV2PI = np.float32(1.0 / TWO_PI)
    consts = (INV2PI, MAGIC, C1, C2, C3)

    def bcast_ap(src, parts):
        return bass.AP(tensor=src.tensor, offset=src.offset, ap=[[0, parts]] + src.ap)

    DEBUG = bool(int(os.environ.get("KERNEL_DEBUG", "0")))
    dbg_done = set()

    def dbg(name, ap):
        if not DEBUG or name in dbg_done:
            return
        dbg_done.add(name)
        t = nc.dram_tensor(f"d_{name}", list(ap.shape), ap.dtype, kind="ExternalOutput").ap()
        nc.sync.dma_start(out=t, in_=ap)

    with tile.TileContext(nc) as tc:
        with (
            tc.tile_pool(name="persist", bufs=1) as persist,
            tc.tile_pool(name="dramp", bufs=1, space="DRAM") as dramp,
        ):
            # small broadcast loads
            qw_sb = persist.tile([P, HD], f32, tag="qw")
            kw_sb = persist.tile([P, HD], f32, tag="kw")
            invf_sb = persist.tile([P, 32], f32, tag="invf")
            nc.gpsimd.dma_start(out=qw_sb, in_=bcast_ap(qw_d, P))
            nc.gpsimd.dma_start(out=kw_sb, in_=bcast_ap(kw_d, P))
            nc.gpsimd.dma_start(out=invf_sb, in_=bcast_ap(invf_dram, P))

            wo_bf = persist.tile([P, 2, DIM], bf16, tag="wo")
            for cc in range(2):
                nc.gpsimd.dma_start(
                    out=wo_bf[:, cc], in_=wo_d[cc * P : (cc + 1) * P, :]
                )

            with tc.tile_pool(name="trigscr", bufs=1) as trigscr:
                tabs_q = _build_trig(
                    nc, tc, persist, trigscr, tpos, qw_sb, invf_sb, consts, "q"
                )
                tabs_k = _build_trig(
                    nc, tc, persist, trigscr, spos, kw_sb, invf_sb, consts, "k"
                )

            kTg = [
                [persist.tile([P, QW], bf16, tag=f"kT{h}_{g}", name=f"kT{h}_{g}") for g in range(QB)]
                for h in range(2)
            ]
            qTg = [
                [persist.tile([P, QW], bf16, tag=f"qT{h}_{g}", name=f"qT{h}_{g}") for g in range(QB)]
                for h in range(2)
            ]
            vaug_flat = persist.tile([P, NT * HPC * (HD + 1) + HD - 1], bf16, tag="vaug")
            nc.vector.memset(vaug_flat, 1.0)
            vaug = vaug_flat[:, 0 : NT * HPC * (HD + 1)].rearrange(
                "p (t h d) -> p t h d", h=HPC, d=HD + 1
            )
            oT = [persist.tile([P, N], bf16, tag=f"oT{i}", name=f"oT{i}") for i in range(2)]

            _spsum_cm = tc.tile_pool(name="spsum", bufs=2, space="PSUM")
            spsum = _spsum_cm.__enter__()
            with (
                tc.tile_pool(name="acts", bufs=1) as acts,
                tc.tile_pool(name="ascr", bufs=2) as ascr,
                tc.tile_pool(name="ppsum", bufs=2, space="PSUM") as ppsum,
            ):
                # ---- input DMAs in (token-block, kc) chunk order: the KV
                # projection's first matmuls only need chunk group 0 ----
                # mega-tile input loads: one DMA descriptor covers all 8 kc
                # blocks (iterating (kc, partition, token) on the DRAM side),
                # split into token halves so the first projection chunks
                # unblock after ~3MB instead of the full 10.5MB; ~6 triggers
                # instead of 32 keeps the trigger queue short
                wkv_all = acts.tile([P, KC, 2 * QD], bf16, tag="wkv")
                xs_all = acts.tile([P, KC, N], bf16, tag="xs")
                xt_all = acts.tile([P, KC, N], bf16, tag="xt")
                wq_all = acts.tile([P, KC, QD], bf16, tag="wq")
                src_r = src_t.rearrange("(k p) n -> p k n", p=P)
                tgt_r = tgt_t.rearrange("(k p) n -> p k n", p=P)
                wkv_r = wkv_d.rearrange("(k p) c -> p k c", p=P)
                HN = N // 2
                # critical first 3MB spread wide: wkv per-kc on the gpsimd
                # trigger queue, xs half-0 16-way on sync; per-queue DMA
                # bandwidth is only ~20GB/s so spread matters more than
                # trigger count for the first chunks
                for kc in range(KC):
                    nc.gpsimd.dma_start(out=wkv_all[:, kc, :], in_=wkv_r[:, kc, :])
                for kc in range(KC):
                    for h in range(2):
                        nc.sync.dma_start(
                            out=xs_all[:, kc, h * HN : (h + 1) * HN],
                            in_=src_r[:, kc, h * HN : (h + 1) * HN],
                        )
                nc.gpsimd.dma_start(
                    out=wq_all, in_=wq_d.rearrange("(k p) c -> p k c", p=P)
                )
                for kc in range(KC):
                    nc.sync.dma_start(out=xt_all[:, kc, 0:HN], in_=tgt_r[:, kc, 0:HN])
                for kc in range(KC):
                    nc.sync.dma_start(out=xt_all[:, kc, HN:N], in_=tgt_r[:, kc, HN:N])
                wkv_bf = [wkv_all[:, kc, :] for kc in range(KC)]
                wq_bf = [wq_all[:, kc, :] for kc in range(KC)]
                xs_bf = [xs_all[:, kc, :] for kc in range(KC)]
                xt_bf = [xt_all[:, kc, :] for kc in range(KC)]

                def xsl(tiles, mc):
                    return [
                        tiles[kc][:, mc * P : (mc + 1) * P] for kc in range(KC)
                    ]

                pt0 = persist.tile([P, NT, 2, QW], bf16, tag="pt0")

                # preload the exp table set during idle ACT time (after trig)
                dummy = acts.tile([P, 1], mybir.dt.float32, tag="dummy")
                nc.vector.memset(dummy, 0.0)
                nc.scalar.activation(dummy, dummy, AF.Exp)

                EPS64 = 64.0 * EPS

                def emit_qk_exp(hp, qb, mc, pt):
                    sp = spsum.tile([P, 2, QW], f32, tag="sstage", name="sp")
                    for i in range(2):
                        pp = slice(i * 64, (i + 1) * 64)
                        nc.tensor.matmul(
                            sp[:, i, :],
                            lhsT=kTg[hp][mc // 4][pp, (mc % 4) * P : (mc % 4 + 1) * P],
                            rhs=qTg[hp][qb][pp, :],
                            start=True,
                            stop=True,
                            tile_position=(i * 64, 0),
                        )
                    nc.scalar.activation(pt[:, mc], sp, AF.Exp)

                # ---- projections, interleaved for the earliest possible
                # exp stream:  phase A = Q chunks 0-7 (tail g0 unblocks
                # qTg[*][0..1]); phase B = KV master loop with Q chunks 8-15
                # woven in, K-side tails every 4 chunks (G=4) so iteration-0
                # QK+exp chunks stream ~6 chunks behind the KV projection ----
                knat = acts.tile([P, NT, QD], bf16, tag="knat")

                kss = persist.tile([P, NT, HPC], f32, tag="kss")
                krsq = persist.tile([P, NT, HPC], f32, tag="krsq")
                qnat = acts.tile([P, NT, QD], bf16, tag="qnat")

                qss = persist.tile([P, NT, HPC], f32, tag="qss")
                qrsq = persist.tile([P, NT, HPC], f32, tag="qrsq")

                def q_chunk(mc):
                    ps = ppsum.tile([P, QD], f32, tag="qps")
                    lhs = xsl(xt_bf, mc)
                    for kc in range(KC):
                        nc.tensor.matmul(
                            ps,
                            lhsT=lhs[kc],
                            rhs=wq_bf[kc],
                            start=(kc == 0),
                            stop=(kc == KC - 1),
                        )
                    nc.scalar.copy(qnat[:, mc], ps)
                    sq = ascr.tile([P, HPC, HD], bf16, tag="qsq", name="sq")
                    nc.scalar.square(sq, ps.rearrange("p (h d) -> p h d", h=HPC))
                    nc.vector.tensor_reduce(
                        qss[:, mc], sq, axis=mybir.AxisListType.X, op=ALU.add
                    )
                    if mc % 4 == 3:
                        # c=1: rsqrt(sumsq+64eps) = rsqrt(ms+eps)/8 folds the
                        # 1/sqrt(hd) score scale into q
                        with tc.high_priority():
                            _tail_group(
                                nc, mc // 4, qss, qrsq, 1.0, qnat, tabs_q,
                                qTg, "q", (acts, ascr), dramp, EPS64, 4,
                            )

                def kv_chunk(mc):
                    ps = ppsum.tile([P, 2 * QD], f32, tag="kvps")
                    lhs = xsl(xs_bf, mc)
                    for kc in range(KC):
                        nc.tensor.matmul(
                            ps,
                            lhsT=lhs[kc],
                            rhs=wkv_bf[kc],
                            start=(kc == 0),
                            stop=(kc == KC - 1),
                        )
                    nc.scalar.copy(knat[:, mc], ps[:, 0:QD])
                    nc.scalar.copy(
                        vaug[:, mc, :, 0:HD],
                        ps[:, QD : 2 * QD].rearrange("p (h d) -> p h d", h=HPC),
                    )
                    sqK = ascr.tile([P, HPC, HD], bf16, tag="ksq", name="sqK")
                    nc.scalar.square(sqK, ps[:, 0:QD].rearrange("p (h d) -> p h d", h=HPC))
                    nc.vector.tensor_reduce(
                        kss[:, mc], sqK, axis=mybir.AxisListType.X, op=ALU.add
                    )
                    if mc % 4 == 3:
                        # c=8: rsq_k = rsqrt(ms+eps) = 8*rsqrt(sumsq+64eps)
                        with tc.high_priority():
                            _tail_group(
                                nc, mc // 4, kss, krsq, 8.0, knat, tabs_k,
                                kTg, "k", (acts, ascr), dramp, EPS64, 4,
                            )

                for mc in range(NT):
                    kv_chunk(mc)
                for mc in range(NT):
                    q_chunk(mc)
                    # head-start: stream iteration-0 scores+exp behind the
                    # Q projection; the K side is fully ready
                    if mc >= 8:
                        emit_qk_exp(0, 0, mc - 8, pt0)
                for amc in range(8, NT):
                    emit_qk_exp(0, 0, amc, pt0)
                dbg("knat", knat)
                dbg("krsq", krsq)
                dbg("qnat", qnat)
                dbg("qrsq", qrsq)

            dbg("vaug", vaug)
            dbg("cw1q", tabs_q[0])
            dbg("sw1q", tabs_q[2])
            # ---- attention + output projection ----
            with (
                tc.tile_pool(name="avpsum", bufs=2, space="PSUM") as avpsum,
                tc.tile_pool(name="ptp", bufs=2) as ptp,
                tc.tile_pool(name="dnp", bufs=4) as dnp,
                tc.tile_pool(name="ostage", bufs=4) as ostage,
            ):
                def emit_wo_tc(qb, ti):
                    t0 = qb * QW + ti * P
                    ost = ostage.tile([P, DIM], f32, tag="ost", name="ost")
                    for od in range(2):
                        wps = avpsum.tile([P, QW], f32, tag=f"av{od}", name="wps")
                        for cc in range(2):
                            nc.tensor.matmul(
                                wps,
                                lhsT=oT[cc][:, t0 : t0 + P],
                                rhs=wo_bf[:, cc, od * 512 : (od + 1) * 512],
                                start=(cc == 0),
                                stop=(cc == 1),
                            )
                        nc.vector.tensor_copy(ost[:, od * 512 : (od + 1) * 512], wps)
                    nc.sync.dma_start(out=out_d[t0 : t0 + P, :], in_=ost)

                pending = []
                its = [(qb, hp) for qb in range(QB) for hp in range(2)]
                pts = {0: pt0}
                # prefetch iteration 1 (qb0, hp1) scores+exp so the exp
                # stream runs dense across the projection/attention seam
                pts[1] = ptp.tile([P, NT, 2, QW], bf16, tag="pt", name="pt1")
                for amc in range(NT):
                    emit_qk_exp(its[1][1], its[1][0], amc, pts[1])
                for idx, (qb, hp) in enumerate(its):
                    pt = pts[idx]
                    av = [
                        avpsum.tile([P, QW], f32, tag=f"av{i}", name=f"av{i}")
                        for i in range(2)
                    ]
                    for mc in range(NT):
                        # produce the NEXT iteration's scores+exp one step
                        # ahead so the ScalarE exp stream never stalls
                        if idx + 1 < len(its) and (idx > 0 or mc < 0):
                            if mc == 0:
                                pts[idx + 1] = ptp.tile(
                                    [P, NT, 2, QW], bf16, tag="pt", name="pt"
                                )
                            nqb, nhp = its[idx + 1]
                            emit_qk_exp(nhp, nqb, mc, pts[idx + 1])
                        if hp == 0 and mc % 4 == 3 and pending:
                            pending.pop(0)()
                        for i in range(2):
                            base = (mc * HPC + 2 * hp + i) * (HD + 1)
                            nc.tensor.matmul(
                                av[i],
                                lhsT=vaug_flat[:, base : base + P],
                                rhs=pt[:, mc, i, :],
                                start=(mc == 0),
                                stop=(mc == NT - 1),
                            )
                        pts.pop(idx - 1, None)
                    if qb == 0 and hp == 0:
                        dbg("pt", pt)
                    for i in range(2):
                        den = dnp.tile([1, QW], f32, tag="den")
                        nc.vector.tensor_copy(den, av[i][HD : HD + 1, :])
                        dn = dnp.tile([1, QW], f32, tag="dn")
                        nc.vector.reciprocal_approx_fast(out=dn, in_=den)
                        dnb = dnp.tile([HD, QW], f32, tag="dnb")
                        nc.gpsimd.partition_broadcast(dnb, dn)
                        nc.vector.tensor_tensor(
                            oT[hp][i * HD : (i + 1) * HD, qb * QW : (qb + 1) * QW],
                            av[i][0:HD, :],
                            dnb,
                            ALU.mult,
                        )
                    if qb == QB - 1:
                        dbg("oT0", oT[0])
                        dbg("oT1", oT[1])
                    # queue this q block's output projection; emitted inside
                    # the next q block's hp0 chunk loop to keep exp fed
                    if hp == 1:
                        pending = [
                            (lambda q, t: lambda: emit_wo_tc(q, t))(qb, ti)
                            for ti in range(QW // P)
                        ]
                with tc.high_priority():
                    for f in pending:
                        f()
            _spsum_cm.__exit__(None, None, None)

    nc.compile()
    return nc


def _get_nc():
    if "nc" not in _CACHE:
        _CACHE["nc"] = _build()
    return _CACHE["nc"]


def _shard(inputs):
    tgt = np.asarray(inputs["tgt"], np.float32)
    src = np.asarray(inputs["src"], np.float32)
    tgt_pos = np.asarray(inputs["tgt_pos"], np.float32)
    src_pos = np.asarray(inputs["src_pos"], np.float32)
    Wq = np.asarray(inputs["Wq"], np.float32)
    Wkv = np.asarray(inputs["Wkv"], np.float32)
    Wo = np.asarray(inputs["Wo"], np.float32)
    qw = np.asarray(inputs["q_norm_w"], np.float32)
    kw = np.asarray(inputs["k_norm_w"], np.float32)

    import ml_dtypes

    bf = ml_dtypes.bfloat16
    in_maps = []
    for c in range(NCORES):
        b, hg = divmod(c, 4)
        cs = slice(hg * QD, (hg + 1) * QD)
        in_maps.append(
            {
                "tgt_t": np.ascontiguousarray(tgt[b].T.astype(bf)),
                "src_t": np.ascontiguousarray(src[b].T.astype(bf)),
                "wq": np.ascontiguousarray(Wq[:, cs].astype(bf)),
                "wkv": np.ascontiguousarray(
                    np.concatenate([Wkv[:, cs], Wkv[:, DIM:][:, cs]], axis=1).astype(bf)
                ),
                "wo": np.ascontiguousarray(Wo[cs, :].astype(bf)),
                "tpos": np.ascontiguousarray(tgt_pos[b]),
                "spos": np.ascontiguousarray(src_pos[b]),
                "qw": np.ascontiguousarray(qw),
                "kw": np.ascontiguousarray(kw),
            }
        )
    return in_maps


def _install_ntff_shim():
    """Provide antenv.axon_hooks (missing in this image) so trace=True can
    capture NTFF profiles through libaxon_pjrt.so."""
    import sys
    import types
    import contextlib
    import ctypes

    if "antenv.axon_hooks" in sys.modules:
        return
    so_path = "/opt/axon/libaxon_pjrt.so"
    if not os.path.exists(so_path):
        return
    lib = ctypes.CDLL(so_path)
    if not hasattr(lib, "axon_start_nrt_profile"):
        return
    lib.axon_start_nrt_profile.argtypes = [
        ctypes.POINTER(ctypes.c_int64),
        ctypes.c_size_t,
    ]
    lib.axon_start_nrt_profile.restype = ctypes.c_int64
    lib.axon_stop_nrt_profile.argtypes = [ctypes.c_char_p]
    lib.axon_stop_nrt_profile.restype = ctypes.c_int64

    @contextlib.contextmanager
    def _hook(output_dir, device_ids):
        import jax

        jax.devices()
        if device_ids:
            ids = (ctypes.c_int64 * len(device_ids))(*device_ids)
            rc = lib.axon_start_nrt_profile(ids, len(device_ids))
        else:
            rc = lib.axon_start_nrt_profile(None, 0)
        if rc != 0:
            raise RuntimeError(f"axon_start_nrt_profile rc={rc}")
        try:
            yield
        finally:
            n = lib.axon_stop_nrt_profile(str(output_dir).encode())
            print(f"ntff profile: {n} file(s) written to {output_dir}")

    mod = types.ModuleType("antenv.axon_hooks")
    mod.get_axon_ntff_profile_hook = lambda: _hook
    mod.set_axon_ntff_profile_hook = lambda h: None
    sys.modules["antenv.axon_hooks"] = mod


def kernel(**inputs) -> np.ndarray:
    global LAST_RESULTS
    from concourse.bass_utils import run_bass_kernel_spmd

    nc = _get_nc()
    in_maps = _shard(inputs)
    trace = bool(int(os.environ.get("KERNEL_TRACE", "0")))
    if trace:
        _install_ntff_shim()
    res = run_bass_kernel_spmd(
        nc, in_maps, core_ids=list(range(NCORES)), trace=trace
    )
    LAST_RESULTS = res
    out = np.zeros((B, N, DIM), np.float32)
    for c in range(NCORES):
        out[c // 4] += res.results[c]["out"]
    return out


# revision 30
# speedup vs baseline: 1.0534x; 1.0534x over previous
"""Trainium2 Bass kernel for cross-attention (b=2, n=m=2048, dim=1024, 16 heads x 64)
with QK-RMSNorm and rotate-half RoPE (float positions), distributed over 8 NeuronCores.

Sharding: core c handles batch b = c//4 and head group hg = c%4 (4 heads each).
Wq/Wkv are column-sharded by head, Wo row-sharded; each core emits a partial
[2048, 1024] output which the host sums over the 4 cores of each batch (the
row-parallel all-reduce done at unshard time).

v2 over the session-1 baseline (245.9us measured fresh / 249.7 re-run):
  - mega-tile input DMAs: one descriptor per (tensor, token-half) iterating
    (kc, partition, token) on the DRAM side -- 6 triggers instead of 32.
    Each sync-queue DMA trigger costs ~0.65us serially, and the projection
    can begin once wkv + the first xs half (3MB) land, so the first matmul
    moves from ~30us to ~15-20us and the exp stream starts ~8us earlier.
  - iteration-1 (qb0, hp1) scores+exp prefetched at the attention-block top
    so the ScalarE exp stream crosses the projection/attention seam with
    zero gaps (measured: the 128-exp stream runs completely dense).
  - norm/rope tails wrapped in tc.high_priority() so the Tile scheduler
    orders them (and their DRAM-bounce + transpose-DMA chains) ahead of the
    bulk projection drains; the K/Q tail scratch (xb/xhat/sq) moved to
    per-tail rotating tiles with per-side tags (shared tags would serialize
    the tails through write-after-read hazards).
  - the timeline is: ~15us DMA/runtime warmup, projections (PE-bound,
    ScalarE absorbs the drain copies+squares since exp hasn't started),
    then a dense 135us exp stream (the wall-setter: 128 x [128,1024] exps
    at (N+352)/1.2ns each) under which QK/AV/Wo matmuls and the softmax
    denominator (ones-row in augmented V, M=65 AV matmuls) all hide.

Rejected experiments (all slower on HW): column-tiled M=64 AV pairs + any
separate denominator path (DVE mc-reduce of P costs 28us/iter strided or
8us/iter as a contiguous add-tree; gpsimd partition_all_reduce 6.7us/iter;
the M=65 ones-row is effectively free since matmul time is set by N, not M),
KV/Q projection interleaving (sync-queue + scheduler serialization), input
chunking into [128,512] tiles (1KB DMA lines + 0.65us/trigger overhead),
Schraudolph fast-exp on DVE for 2/16 chunks (pipeline disruption), custom-DVE
ops at partition base 64 (silently wrong on HW - reciprocal must run at
base 0 with a cross-base normalize, which standard DVE ops do support).
"""

import contextlib
import math
import os

import numpy as np

B, N, DIM, H, HD = 2, 2048, 1024, 16, 64
NCORES = 8
HPC = 4  # heads per core
QD = HPC * HD  # 256
P = 128
NT = N // P  # 16 token chunks
KC = DIM // P  # 8 contraction chunks
QB = 4  # q blocks of 512
QW = N // QB  # 512
G = 8  # token chunks per norm/rope tail group
ROPE_THETA = 10000.0
EPS = float(np.finfo(np.float32).eps)

_CACHE = {}
LAST_RESULTS = None


def _build_trig(nc, tc, pool, scr, pos_dram, w_sb, invf_sb, consts, tag):
    """sin/cos tables with RMSNorm-weight w folded in. Returns (cw1, cw2, sw1, sw2),
    each [P, NT, 32] bf16: cw1=cos*w[0:32], cw2=cos*w[32:64], sw1=sin*w[0:32],
    sw2=sin*w[32:64]."""
    import concourse.bass as bass
    from concourse import mybir

    f32 = mybir.dt.float32
    bf16 = mybir.dt.bfloat16
    AF = mybir.ActivationFunctionType
    ALU = mybir.AluOpType
    INV2PI, MAGIC, C1, C2, C3 = consts

    pos_sb = scr.tile([P, NT], f32, tag=f"pos{tag}")
    nc.sync.dma_start(out=pos_sb, in_=pos_dram.rearrange("(c p) -> p c", p=P))
    ang = scr.tile([P, NT * 32], f32, tag=f"ang{tag}")
    nc.vector.tensor_tensor(
        ang.rearrange("p (t j) -> p t j", j=32),
        pos_sb[:, :, None].to_broadcast([P, NT, 32]),
        invf_sb[:, None, :].to_broadcast([P, NT, 32]),
        ALU.mult,
    )
    # round(ang / 2pi) via magic-number rounding
    kf = scr.tile([P, NT * 32], f32, tag=f"kf{tag}")
    nc.vector.tensor_scalar(kf, ang, float(INV2PI), float(MAGIC), ALU.mult, ALU.add)
    nc.vector.tensor_scalar(kf, kf, float(MAGIC), None, ALU.subtract)
    angr = scr.tile([P, NT * 32], f32, tag=f"angr{tag}")
    nc.vector.cody_waite_cascade(
        out=angr, x=ang, k=kf, c1=float(C1), c2=float(C2), c3=float(C3)
    )
    # cos argument: wrap(angr + pi/2) into [-pi, pi]
    nc.vector.add_range_wrap(
        out=kf, in_=angr, shift=math.pi / 2, bound=math.pi, period=2 * math.pi
    )
    sint = scr.tile([P, NT, 32], f32, tag=f"sin{tag}")
    cost = scr.tile([P, NT, 32], f32, tag=f"cos{tag}")
    nc.scalar.activation(sint.rearrange("p t j -> p (t j)"), angr, AF.Sin)
    nc.scalar.activation(cost.rearrange("p t j -> p (t j)"), kf, AF.Sin)

    tabs = []
    for name, trig, wlo in (
        ("cw1", cost, True),
        ("cw2", cost, False),
        ("sw1", sint, True),
        ("sw2", sint, False),
    ):
        t = pool.tile([P, NT, 32], bf16, tag=f"{name}{tag}")
        wsl = w_sb[:, 0:32] if wlo else w_sb[:, 32:64]
        nc.vector.tensor_tensor(
            t, trig, wsl[:, None, :].to_broadcast([P, NT, 32]), ALU.mult
        )
        tabs.append(t)
    return tabs


def _tail_group(nc, g, ss, rsq, c, xnat, tabs, xTg, nm, pools, dramp, eps64, gg):
    """After projection chunks gg*g..gg*g+gg-1: rsqrt (DVE fast-inverse-sqrt, 2 NR
    steps), normalize+rope the group, bounce to DRAM, transposed-load into
    xTg[half][(gg//4)*g + sub]."""
    from concourse import mybir

    f32 = mybir.dt.float32
    i32 = mybir.dt.int32
    bf16 = mybir.dt.bfloat16
    ALU = mybir.AluOpType
    acts, ascr = pools
    cw1, cw2, sw1, sw2 = tabs
    MAGIC = 0x5F3759DF
    gs = slice(gg * g, gg * g + gg)

    xg = ascr.tile([P, gg, HPC], f32, tag=f"rsx{gg}", name="rsx")
    yg = ascr.tile([P, gg, HPC], f32, tag=f"rsy{gg}", name="rsy")
    tg = ascr.tile([P, gg, HPC], f32, tag=f"rst{gg}", name="rst")
    nc.vector.tensor_scalar_add(xg, ss[:, gs], eps64)
    nc.vector.tensor_scalar(
        tg.bitcast(i32), xg.bitcast(i32), 1, None, ALU.arith_shift_right
    )
    nc.vector.tensor_scalar(
        yg.bitcast(i32), tg.bitcast(i32), -1, MAGIC, ALU.mult, ALU.add
    )
    nc.vector.tensor_tensor(tg, yg, yg, ALU.mult)
    nc.vector.tensor_tensor(tg, tg, xg, ALU.mult)
    nc.vector.tensor_scalar(tg, tg, -0.5, 1.5, ALU.mult, ALU.add)
    nc.vector.tensor_tensor(yg, yg, tg, ALU.mult)
    nc.vector.tensor_tensor(tg, yg, yg, ALU.mult)
    nc.vector.tensor_tensor(tg, tg, xg, ALU.mult)
    nc.vector.tensor_scalar(tg, tg, -0.5 * c, 1.5 * c, ALU.mult, ALU.add)
    nc.vector.tensor_tensor(rsq[:, gs], yg, tg, ALU.mult)

    xb = ascr.tile([P, gg, HPC, HD], bf16, tag=f"xb{nm}", name=f"xb{nm}")
    xhat = ascr.tile([P, gg, HPC, 2, 32], bf16, tag=f"xh{nm}", name=f"xh{nm}")
    x4 = xnat.rearrange("p t (h d) -> p t h d", h=HPC)
    nc.vector.tensor_tensor(
        xb,
        x4[:, gs],
        rsq[:, gs, :, None].to_broadcast([P, gg, HPC, HD]),
        ALU.mult,
    )
    x1 = xb[:, :, :, 0:32]
    x2 = xb[:, :, :, 32:64]
    sh4 = [P, gg, HPC, 32]

    def bc(t):
        return t[:, gs, None, :].to_broadcast(sh4)

    a = ascr.tile(sh4, bf16, tag=f"ra{gg}", name="ra")
    b = ascr.tile(sh4, bf16, tag=f"rb{gg}", name="rb")
    nc.vector.tensor_tensor(a, x1, bc(cw1), ALU.mult)
    nc.vector.tensor_tensor(b, x2, bc(sw2), ALU.mult)
    nc.vector.tensor_sub(xhat[:, :, :, 0, :], a, b)
    a2 = ascr.tile(sh4, bf16, tag=f"ra{gg}", name="ra2")
    b2 = ascr.tile(sh4, bf16, tag=f"rb{gg}", name="rb2")
    nc.vector.tensor_tensor(a2, x1, bc(sw1), ALU.mult)
    nc.vector.tensor_tensor(b2, x2, bc(cw2), ALU.mult)
    nc.vector.tensor_add(xhat[:, :, :, 1, :], a2, b2)

    scr = dramp.tile([gg * P, QD], bf16, tag=f"scr{nm}{g}", name=f"scr{nm}{g}")
    nc.sync.dma_start(
        out=scr.rearrange("(c p) d -> p c d", p=P),
        in_=xhat.rearrange("p t h two j -> p t (h two j)"),
    )
    nsub = (gg * P) // QW
    for half in range(2):
        for sub in range(nsub):
            nc.sync.dma_start(
                out=xTg[half][(gg // 4) * g + sub],
                in_=scr[sub * QW : (sub + 1) * QW, half * P : (half + 1) * P],
                transpose=True,
            )


def _build():
    import concourse.bass as bass
    import concourse.tile as tile
    from concourse import bacc, bass_isa, mybir

    f32 = mybir.dt.float32
    bf16 = mybir.dt.bfloat16
    AF = mybir.ActivationFunctionType
    ALU = mybir.AluOpType

    nc = bacc.Bacc(
        "TRN2", target_bir_lowering=False, debug=False, num_devices=NCORES
    )

    tgt_t = nc.dram_tensor("tgt_t", [DIM, N], bf16, kind="ExternalInput").ap()
    src_t = nc.dram_tensor("src_t", [DIM, N], bf16, kind="ExternalInput").ap()
    wq_d = nc.dram_tensor("wq", [DIM, QD], bf16, kind="ExternalInput").ap()
    wkv_d = nc.dram_tensor("wkv", [DIM, 2 * QD], bf16, kind="ExternalInput").ap()
    wo_d = nc.dram_tensor("wo", [QD, DIM], bf16, kind="ExternalInput").ap()
    tpos = nc.dram_tensor("tpos", [N], f32, kind="ExternalInput").ap()
    spos = nc.dram_tensor("spos", [N], f32, kind="ExternalInput").ap()
    qw_d = nc.dram_tensor("qw", [HD], f32, kind="ExternalInput").ap()
    kw_d = nc.dram_tensor("kw", [HD], f32, kind="ExternalInput").ap()
    out_d = nc.dram_tensor("out", [N, DIM], f32, kind="ExternalOutput").ap()

    invf_np = np.float32(ROPE_THETA) ** (
        -np.arange(0, HD, 2, dtype=np.float32) / np.float32(HD)
    )
    invf_dram = nc.inline_tensor(invf_np.astype(np.float32), "invf").ap()

    TWO_PI = 2 * math.pi
    C1 = np.float32(6.28125)
    C2 = np.float32(TWO_PI - float(C1))
    C3 = np.float32(TWO_PI - float(C1) - float(C2))
    MAGIC = np.float32(1.5 * 2**23)
    INV2PI = np.float32(1.0 / TWO_PI)
    consts = (INV2PI, MAGIC, C1, C2, C3)

    def bcast_ap(src, parts):
        return bass.AP(tensor=src.tensor, offset=src.offset, ap=[[0, parts]] + src.ap)

    DEBUG = bool(int(os.environ.get("KERNEL_DEBUG", "0")))
    dbg_done = set()

    def dbg(name, ap):
        if not DEBUG or name in dbg_done:
            return
        dbg_done.add(name)
        t = nc.dram_tensor(f"d_{name}", list(ap.shape), ap.dtype, kind="ExternalOutput").ap()
        nc.sync.dma_start(out=t, in_=ap)

    with tile.TileContext(nc) as tc:
        with (
            tc.tile_pool(name="persist", bufs=1) as persist,
            tc.tile_pool(name="dramp", bufs=1, space="DRAM") as dramp,
        ):
            # small broadcast loads
            qw_sb = persist.tile([P, HD], f32, tag="qw")
            kw_sb = persist.tile([P, HD], f32, tag="kw")
            invf_sb = persist.tile([P, 32], f32, tag="invf")
            nc.gpsimd.dma_start(out=qw_sb, in_=bcast_ap(qw_d, P))
            nc.gpsimd.dma_start(out=kw_sb, in_=bcast_ap(kw_d, P))
            nc.gpsimd.dma_start(out=invf_sb, in_=bcast_ap(invf_dram, P))

            wo_bf = persist.tile([P, 2, DIM], bf16, tag="wo")
            for cc in range(2):
                nc.gpsimd.dma_start(
                    out=wo_bf[:, cc], in_=wo_d[cc * P : (cc + 1) * P, :]
                )

            with tc.tile_pool(name="trigscr", bufs=1) as trigscr:
                tabs_q = _build_trig(
                    nc, tc, persist, trigscr, tpos, qw_sb, invf_sb, consts, "q"
                )
                tabs_k = _build_trig(
                    nc, tc, persist, trigscr, spos, kw_sb, invf_sb, consts, "k"
                )

            kTg = [
                [persist.tile([P, QW], bf16, tag=f"kT{h}_{g}", name=f"kT{h}_{g}") for g in range(QB)]
                for h in range(2)
            ]
            qTg = [
                [persist.tile([P, QW], bf16, tag=f"qT{h}_{g}", name=f"qT{h}_{g}") for g in range(QB)]
                for h in range(2)
            ]
            vaug_flat = persist.tile([P, NT * HPC * (HD + 1) + HD - 1], bf16, tag="vaug")
            nc.vector.memset(vaug_flat, 1.0)
            vaug = vaug_flat[:, 0 : NT * HPC * (HD + 1)].rearrange(
                "p (t h d) -> p t h d", h=HPC, d=HD + 1
            )
            oT = [persist.tile([P, N], bf16, tag=f"oT{i}", name=f"oT{i}") for i in range(2)]

            _spsum_cm = tc.tile_pool(name="spsum", bufs=2, space="PSUM")
            spsum = _spsum_cm.__enter__()
            with (
                tc.tile_pool(name="acts", bufs=1) as acts,
                tc.tile_pool(name="ascr", bufs=2) as ascr,
                tc.tile_pool(name="ppsum", bufs=2, space="PSUM") as ppsum,
            ):
                # ---- input DMAs in (token-block, kc) chunk order: the KV
                # projection's first matmuls only need chunk group 0 ----
                # mega-tile input loads: one DMA descriptor covers all 8 kc
                # blocks (iterating (kc, partition, token) on the DRAM side),
                # split into token halves so the first projection chunks
                # unblock after ~3MB instead of the full 10.5MB; ~6 triggers
                # instead of 32 keeps the trigger queue short
                wkv_all = acts.tile([P, KC, 2 * QD], bf16, tag="wkv")
                xs_all = acts.tile([P, KC, N], bf16, tag="xs")
                xt_all = acts.tile([P, KC, N], bf16, tag="xt")
                wq_all = acts.tile([P, KC, QD], bf16, tag="wq")
                src_r = src_t.rearrange("(k p) n -> p k n", p=P)
                tgt_r = tgt_t.rearrange("(k p) n -> p k n", p=P)
                HN = N // 2
                wkv_r = wkv_d.rearrange("(k p) c -> p k c", p=P)
                HK = KC // 2
                nc.sync.dma_start(
                    out=wq_all, in_=wq_d.rearrange("(k p) c -> p k c", p=P)
                )
                QN = N // 4
                nc.sync.dma_start(out=xt_all[:, 0:HK, 0:QN], in_=tgt_r[:, 0:HK, 0:QN])
                nc.sync.dma_start(out=xt_all[:, HK:KC, 0:QN], in_=tgt_r[:, HK:KC, 0:QN])
                nc.sync.dma_start(out=xt_all[:, 0:HK, QN:HN], in_=tgt_r[:, 0:HK, QN:HN])
                nc.sync.dma_start(out=xt_all[:, HK:KC, QN:HN], in_=tgt_r[:, HK:KC, QN:HN])
                nc.sync.dma_start(out=wkv_all[:, 0:HK, :], in_=wkv_r[:, 0:HK, :])
                nc.sync.dma_start(out=wkv_all[:, HK:KC, :], in_=wkv_r[:, HK:KC, :])
                nc.sync.dma_start(out=xs_all[:, 0:HK, 0:HN], in_=src_r[:, 0:HK, 0:HN])
                nc.sync.dma_start(out=xs_all[:, HK:KC, 0:HN], in_=src_r[:, HK:KC, 0:HN])
                nc.sync.dma_start(out=xs_all[:, :, HN:N], in_=src_r[:, :, HN:N])
                nc.sync.dma_start(out=xt_all[:, :, HN:N], in_=tgt_r[:, :, HN:N])
                wkv_bf = [wkv_all[:, kc, :] for kc in range(KC)]
                wq_bf = [wq_all[:, kc, :] for kc in range(KC)]
                xs_bf = [xs_all[:, kc, :] for kc in range(KC)]
                xt_bf = [xt_all[:, kc, :] for kc in range(KC)]

                def xsl(tiles, mc):
                    return [
                        tiles[kc][:, mc * P : (mc + 1) * P] for kc in range(KC)
                    ]

                pt0 = persist.tile([P, NT, 2, QW], bf16, tag="pt0")

                # preload the exp table set during idle ACT time (after trig)
                dummy = acts.tile([P, 1], mybir.dt.float32, tag="dummy")
                nc.vector.memset(dummy, 0.0)
                nc.scalar.activation(dummy, dummy, AF.Exp)

                EPS64 = 64.0 * EPS

                def emit_qk_exp(hp, qb, mc, pt):
                    sp = spsum.tile([P, 2, QW], f32, tag="sstage", name="sp")
                    for i in range(2):
                        pp = slice(i * 64, (i + 1) * 64)
                        nc.tensor.matmul(
                            sp[:, i, :],
                            lhsT=kTg[hp][mc // 4][pp, (mc % 4) * P : (mc % 4 + 1) * P],
                            rhs=qTg[hp][qb][pp, :],
                            start=True,
                            stop=True,
                            tile_position=(i * 64, 0),
                        )
                    nc.scalar.activation(pt[:, mc], sp, AF.Exp)

                # ---- projections, interleaved for the earliest possible
                # exp stream:  phase A = Q chunks 0-7 (tail g0 unblocks
                # qTg[*][0..1]); phase B = KV master loop with Q chunks 8-15
                # woven in, K-side tails every 4 chunks (G=4) so iteration-0
                # QK+exp chunks stream ~6 chunks behind the KV projection ----
                knat = acts.tile([P, NT, QD], bf16, tag="knat")

                kss = persist.tile([P, NT, HPC], f32, tag="kss")
                krsq = persist.tile([P, NT, HPC], f32, tag="krsq")
                qnat = acts.tile([P, NT, QD], bf16, tag="qnat")

                qss = persist.tile([P, NT, HPC], f32, tag="qss")
                qrsq = persist.tile([P, NT, HPC], f32, tag="qrsq")

                def q_chunk(mc):
                    ps = ppsum.tile([P, QD], f32, tag="qps")
                    lhs = xsl(xt_bf, mc)
                    for kc in range(KC):
                        nc.tensor.matmul(
                            ps,
                            lhsT=lhs[kc],
                            rhs=wq_bf[kc],
                            start=(kc == 0),
                            stop=(kc == KC - 1),
                        )
                    nc.scalar.copy(qnat[:, mc], ps)
                    sq = ascr.tile([P, HPC, HD], bf16, tag="qsq", name="sq")
                    nc.scalar.square(sq, ps.rearrange("p (h d) -> p h d", h=HPC))
                    nc.vector.tensor_reduce(
                        qss[:, mc], sq, axis=mybir.AxisListType.X, op=ALU.add
                    )
                    if mc % 4 == 3:
                        # c=1: rsqrt(sumsq+64eps) = rsqrt(ms+eps)/8 folds the
                        # 1/sqrt(hd) score scale into q
                        with tc.high_priority():
                            _tail_group(
                                nc, mc // 4, qss, qrsq, 1.0, qnat, tabs_q,
                                qTg, "q", (acts, ascr), dramp, EPS64, 4,
                            )

                def kv_chunk(mc):
                    ps = ppsum.tile([P, 2 * QD], f32, tag="kvps")
                    lhs = xsl(xs_bf, mc)
                    for kc in range(KC):
                        nc.tensor.matmul(
                            ps,
                            lhsT=lhs[kc],
                            rhs=wkv_bf[kc],
                            start=(kc == 0),
                            stop=(kc == KC - 1),
                        )
                    nc.scalar.copy(knat[:, mc], ps[:, 0:QD])
                    nc.scalar.copy(
                        vaug[:, mc, :, 0:HD],
                        ps[:, QD : 2 * QD].rearrange("p (h d) -> p h d", h=HPC),
                    )
                    sqK = ascr.tile([P, HPC, HD], bf16, tag="ksq", name="sqK")
                    nc.scalar.square(sqK, ps[:, 0:QD].rearrange("p (h d) -> p h d", h=HPC))
                    nc.vector.tensor_reduce(
                        kss[:, mc], sqK, axis=mybir.AxisListType.X, op=ALU.add
                    )
                    if mc % 4 == 3:
                        # c=8: rsq_k = rsqrt(ms+eps) = 8*rsqrt(sumsq+64eps)
                        with tc.high_priority():
                            _tail_group(
                                nc, mc // 4, kss, krsq, 8.0, knat, tabs_k,
                                kTg, "k", (acts, ascr), dramp, EPS64, 4,
                            )

                # Q prefix: chunks 0-3 + tail g0 unblock qTg[*][0], so
                # iteration-0 exps can stream DURING the KV projection,
                # paced ~6 chunks behind the K-side G=4 tails
                for mc in range(4):
                    q_chunk(mc)
                for m in range(NT):
                    kv_chunk(m)
                    if m >= 6:
                        emit_qk_exp(0, 0, m - 6, pt0)
                for mc in range(4, NT):
                    q_chunk(mc)
                    if 6 <= mc <= 11:
                        emit_qk_exp(0, 0, mc + 4, pt0)
                dbg("knat", knat)
                dbg("krsq", krsq)
                dbg("qnat", qnat)
                dbg("qrsq", qrsq)

            dbg("vaug", vaug)
            dbg("cw1q", tabs_q[0])
            dbg("sw1q", tabs_q[2])
            # ---- attention + output projection ----
            with (
                tc.tile_pool(name="avpsum", bufs=2, space="PSUM") as avpsum,
                tc.tile_pool(name="ptp", bufs=2) as ptp,
                tc.tile_pool(name="dnp", bufs=4) as dnp,
                tc.tile_pool(name="ostage", bufs=4) as ostage,
            ):
                def emit_wo_tc(qb, ti):
                    t0 = qb * QW + ti * P
                    ost = ostage.tile([P, DIM], f32, tag="ost", name="ost")
                    for od in range(2):
                        wps = avpsum.tile([P, QW], f32, tag=f"av{od}", name="wps")
                        for cc in range(2):
                            nc.tensor.matmul(
                                wps,
                                lhsT=oT[cc][:, t0 : t0 + P],
                                rhs=wo_bf[:, cc, od * 512 : (od + 1) * 512],
                                start=(cc == 0),
                                stop=(cc == 1),
                            )
                        nc.vector.tensor_copy(ost[:, od * 512 : (od + 1) * 512], wps)
                    nc.sync.dma_start(out=out_d[t0 : t0 + P, :], in_=ost)

                # for the final q block: oT[0][:, qb3] is complete one
                # iteration early, and the spsum ring idles during the last
                # iteration (no more exps) -- pre-accumulate the cc=0 half of
                # three Wo blocks there so the PE stays HAM-warm through the
                # last denominator chain and only cc=1 remains for the tail
                pre_wps = {}

                def pre_wo_cc0(qb, ti):
                    t0 = qb * QW + ti * P
                    if ti < 2:
                        w2 = spsum.tile([P, 2, QW], f32, tag="sstage", name=f"wopre{ti}")
                        pair = (w2[:, 0, :], w2[:, 1, :])
                    else:
                        pair = (
                            avpsum.tile([P, QW], f32, tag="av0", name="wpre0"),
                            avpsum.tile([P, QW], f32, tag="av1", name="wpre1"),
                        )
                    pre_wps[ti] = pair
                    for od in range(2):
                        nc.tensor.matmul(
                            pair[od],
                            lhsT=oT[0][:, t0 : t0 + P],
                            rhs=wo_bf[:, 0, od * 512 : (od + 1) * 512],
                            start=True,
                            stop=False,
                            skip_group_check=True,
                        )

                def finish_wo(qb, ti):
                    t0 = qb * QW + ti * P
                    pair = pre_wps[ti]
                    ost = ostage.tile([P, DIM], f32, tag="ost", name="ost")
                    for od in range(2):
                        nc.tensor.matmul(
                            pair[od],
                            lhsT=oT[1][:, t0 : t0 + P],
                            rhs=wo_bf[:, 1, od * 512 : (od + 1) * 512],
                            start=False,
                            stop=True,
                            skip_group_check=True,
                        )
                        nc.vector.tensor_copy(ost[:, od * 512 : (od + 1) * 512], pair[od])
                    nc.sync.dma_start(out=out_d[t0 : t0 + P, :], in_=ost)

                pending = []
                its = [(qb, hp) for qb in range(QB) for hp in range(2)]
                pts = {0: pt0}
                # prefetch iteration 1 (qb0, hp1) scores+exp so the exp
                # stream runs dense across the projection/attention seam
                pts[1] = ptp.tile([P, NT, 2, QW], bf16, tag="pt", name="pt1")
                for amc in range(NT):
                    emit_qk_exp(its[1][1], its[1][0], amc, pts[1])
                for idx, (qb, hp) in enumerate(its):
                    pt = pts[idx]
                    av = [
                        avpsum.tile([P, QW], f32, tag=f"av{i}", name=f"av{i}")
                        for i in range(2)
                    ]
                    for mc in range(NT):
                        # produce the NEXT iteration's scores+exp one step
                        # ahead so the ScalarE exp stream never stalls
                        if idx + 1 < len(its) and (idx > 0 or mc < 0):
                            if mc == 0:
                                pts[idx + 1] = ptp.tile(
                                    [P, NT, 2, QW], bf16, tag="pt", name="pt"
                                )
                            nqb, nhp = its[idx + 1]
                            emit_qk_exp(nhp, nqb, mc, pts[idx + 1])
                        if hp == 0 and mc % 4 == 3 and pending:
                            pending.pop(0)()
                        if qb == QB - 1 and hp == 1 and mc in (2, 6, 10):
                            pre_wo_cc0(qb, {2: 0, 6: 1, 10: 2}[mc])
                        for i in range(2):
                            base = (mc * HPC + 2 * hp + i) * (HD + 1)
                            nc.tensor.matmul(
                                av[i],
                                lhsT=vaug_flat[:, base : base + P],
                                rhs=pt[:, mc, i, :],
                                start=(mc == 0),
                                stop=(mc == NT - 1),
                            )
                        pts.pop(idx - 1, None)
                    if qb == 0 and hp == 0:
                        dbg("pt", pt)
                    den_cm = (
                        tc.high_priority()
                        if idx == len(its) - 1
                        else contextlib.nullcontext()
                    )
                    with den_cm:
                        for i in range(2):
                            den = dnp.tile([1, QW], f32, tag="den")
                            nc.vector.tensor_copy(den, av[i][HD : HD + 1, :])
                            dn = dnp.tile([1, QW], f32, tag="dn")
                            nc.vector.reciprocal_approx_fast(out=dn, in_=den)
                            dnb = dnp.tile([HD, QW], f32, tag="dnb")
                            nc.gpsimd.partition_broadcast(dnb, dn)
                            nc.vector.tensor_tensor(
                                oT[hp][i * HD : (i + 1) * HD, qb * QW : (qb + 1) * QW],
                                av[i][0:HD, :],
                                dnb,
                                ALU.mult,
                            )
                    if qb == QB - 1:
                        dbg("oT0", oT[0])
                        dbg("oT1", oT[1])
                    # queue this q block's output projection; emitted inside
                    # the next q block's hp0 chunk loop to keep exp fed
                    if hp == 1:
                        if qb == QB - 1:
                            pending = [
                                (lambda t: lambda: finish_wo(QB - 1, t))(ti)
                                for ti in range(3)
                            ] + [lambda: emit_wo_tc(QB - 1, 3)]
                        else:
                            pending = [
                                (lambda q, t: lambda: emit_wo_tc(q, t))(qb, ti)
                                for ti in range(QW // P)
                            ]
                with tc.high_priority():
                    for f in pending:
                        f()
            _spsum_cm.__exit__(None, None, None)

    nc.compile()
    return nc


def _get_nc():
    if "nc" not in _CACHE:
        _CACHE["nc"] = _build()
    return _CACHE["nc"]


def _shard(inputs):
    tgt = np.asarray(inputs["tgt"], np.float32)
    src = np.asarray(inputs["src"], np.float32)
    tgt_pos = np.asarray(inputs["tgt_pos"], np.float32)
    src_pos = np.asarray(inputs["src_pos"], np.float32)
    Wq = np.asarray(inputs["Wq"], np.float32)
    Wkv = np.asarray(inputs["Wkv"], np.float32)
    Wo = np.asarray(inputs["Wo"], np.float32)
    qw = np.asarray(inputs["q_norm_w"], np.float32)
    kw = np.asarray(inputs["k_norm_w"], np.float32)

    import ml_dtypes

    bf = ml_dtypes.bfloat16
    in_maps = []
    for c in range(NCORES):
        b, hg = divmod(c, 4)
        cs = slice(hg * QD, (hg + 1) * QD)
        in_maps.append(
            {
                "tgt_t": np.ascontiguousarray(tgt[b].T.astype(bf)),
                "src_t": np.ascontiguousarray(src[b].T.astype(bf)),
                "wq": np.ascontiguousarray(Wq[:, cs].astype(bf)),
                "wkv": np.ascontiguousarray(
                    np.concatenate([Wkv[:, cs], Wkv[:, DIM:][:, cs]], axis=1).astype(bf)
                ),
                "wo": np.ascontiguousarray(Wo[cs, :].astype(bf)),
                "tpos": np.ascontiguousarray(tgt_pos[b]),
                "spos": np.ascontiguousarray(src_pos[b]),
                "qw": np.ascontiguousarray(qw),
                "kw": np.ascontiguousarray(kw),
            }
        )
    return in_maps


def _install_ntff_shim():
    """Provide antenv.axon_hooks (missing in this image) so trace=True can
    capture NTFF profiles through libaxon_pjrt.so."""
    import sys
    import types
    import contextlib
    import ctypes

    if "antenv.axon_hooks" in sys.modules:
        return
    so_path = "/opt/axon/libaxon_pjrt.so"
    if not os.path.exists(so_path):
        return
    lib = ctypes.CDLL(so_path)
    if not hasattr(lib, "axon_start_nrt_profile"):
        return
    lib.axon_start_nrt_profile.argtypes = [
        ctypes.POINTER(ctypes.c_int64),
        ctypes.c_size_t,
    ]
    lib.axon_start_nrt_profile.restype = ctypes.c_int64
    lib.axon_stop_nrt_profile.argtypes = [ctypes.c_char_p]
    lib.axon_stop_nrt_profile.restype = ctypes.c_int64

    @contextlib.contextmanager
    def _hook(output_dir, device_ids):
        import jax

        jax.devices()
        if device_ids:
            ids = (ctypes.c_int64 * len(device_ids))(*device_ids)
            rc = lib.axon_start_nrt_profile(ids, len(device_ids))
        else:
            rc = lib.axon_start_nrt_profile(None, 0)
        if rc != 0:
            raise RuntimeError(f"axon_start_nrt_profile rc={rc}")
        try:
            yield
        finally:
            n = lib.axon_stop_nrt_profile(str(output_dir).encode())
            print(f"ntff profile: {n} file(s) written to {output_dir}")

    mod = types.ModuleType("antenv.axon_hooks")
    mod.get_axon_ntff_profile_hook = lambda: _hook
    mod.set_axon_ntff_profile_hook = lambda h: None
    sys.modules["antenv.axon_hooks"] = mod


def kernel(**inputs) -> np.ndarray:
    global LAST_RESULTS
    from concourse.bass_utils import run_bass_kernel_spmd

    nc = _get_nc()
    in_maps = _shard(inputs)
    trace = bool(int(os.environ.get("KERNEL_TRACE", "0")))
    if trace:
        _install_ntff_shim()
    res = run_bass_kernel_spmd(
        nc, in_maps, core_ids=list(range(NCORES)), trace=trace
    )
    LAST_RESULTS = res
    out = np.zeros((B, N, DIM), np.float32)
    for c in range(NCORES):
        out[c // 4] += res.results[c]["out"]
    return out
